# revision 70
# baseline (speedup 1.0000x reference)
"""JSD loss kernel for Trainium2 (8 NeuronCores, token-sharded SPMD).

Transposed ("vocab-on-partitions") design: the host pre-transposes each
core's [256 tok, 32000 voc] shard to fp16 lp3 = lp + (B+1)*ln2 (B=15)
stored block-interleaved ([vblock, lp|lq, 128, 256tok] rows), so one
DMA per super-tile lands the [lp_b|lq_b|...] pair in SBUF.  Per
128-vocab-row super-tile of G blocks (free width W = G*256, pair 2W):

  ACT Exp -> pq (2W cols) ; m = p'+q' in SBUF fp16 (Pool/DVE split,
  capped by the next tile's Exp window) ; ACT Ln -> lnm (W cols) ;
  DVE prod1 = pq*lplq in-place (no Ln dep: frees the load buffer
  fast) and prod2 = pq*lnm (lnm broadcast over the pair via a
  zero-stride dim) - both plain fp16 TTs in DVE 2x mode ; PE ones-
  matmul (+/-2^-16 fp16-subnormal lhsT, exact) reduces each product
  over partitions into two [1,256] PSUM chains accumulated across the
  whole stream.  loss = acc1 + acc2 (the +(B+1)ln2 bias cancels).

The per-token vocab sum thus lands on the PE (otherwise idle) and DVE
runs only 2x-mode TTs - no 1x STT/TensorReduce/Pool folds exist.  ACT
is the sole bottleneck at its floor (3V cols/block: Exp 2V + Ln V =
160us + bubbles).  Tile sizes taper at both ends (fast DMA fill, short
drain); the last tiles use dedicated prefetched lplq/pq buffers so the
drain never waits on DMA or PE buffer recycling.  TimelineSim:
183,184 ns/core (baseline row-layout kernel: 219,348 ns).
"""

import sys
from contextlib import ExitStack

import numpy as np

sys.path.insert(0, "/opt/trn_rl_repo")

N, V = 2048, 32000
NCORES = 8
R = N // NCORES  # 256 tokens per core, all on the free dim
P = 128
NVB = V // P  # 250 vocab partition-blocks per core
# super-tile sizes in vocab blocks: tapered head (fast fill) and tail
# (fast drain, small PE backlog) around full G=16 bodies; sum must be 250
TILES = [2, 2, 4, 8, 12] + [16] * 13 + [6, 4, 3, 1]
POOL_SHARE = 0.55  # fraction of each m-add done on Pool (rest on DVE)
BIAS = 15.0
LN2 = 0.6931471805599453

_CACHE = {}


def _preload_act_table(nc):
    """Preload the act-func table containing BOTH Exp and Ln so the
    insert_act_table_loads pass never thrashes tables (1283 ns each)."""
    from concourse import mybir
    from concourse.hw_specs import get_activation_tables

    tabs = get_activation_tables(nc.m.arch)
    E = mybir.ActivationFunctionType
    for i, (name, funcs) in enumerate(tabs.items()):
        if E.Exp in funcs and E.Ln in funcs:
            inst = mybir.InstLoadActFuncSet(
                name=nc.get_next_instruction_name(),
                ins=[],
                outs=[],
                act_func_set_id=i,
            )
            inst.engine = mybir.EngineType.Activation
            nc.scalar.add_instruction(inst)
            return


def _build_program():
    import concourse.bacc as bacc
    import concourse.tile as tile
    from concourse import mybir

    nc = bacc.Bacc(
        "TRN2",
        target_bir_lowering=False,
        debug=False,
        enable_asserts=False,
        num_devices=1,
    )
    lpq_d = nc.dram_tensor(
        "lpq3", [2 * V, R], mybir.dt.float16, kind="ExternalInput"
    )
    out_d = nc.dram_tensor("loss", [1, R], mybir.dt.float32, kind="ExternalOutput")

    # host layout: row index = (vblock * 2 + a) * 128 + p  (a = lp/lq)
    lpq = lpq_d.ap()

    fp32 = mybir.dt.float32
    fp16 = mybir.dt.float16
    Exp = mybir.ActivationFunctionType.Exp
    Ln = mybir.ActivationFunctionType.Ln
    mult = mybir.AluOpType.mult
    add = mybir.AluOpType.add
    subtract = mybir.AluOpType.subtract

    GMAX = max(TILES)
    WMAX = GMAX * R
    NT = len(TILES)
    NRED = sum(2 * g for g in TILES)  # reduce-matmuls total

    with tile.TileContext(nc) as tc, ExitStack() as ctx:
        _preload_act_table(nc)
        const = ctx.enter_context(tc.tile_pool(name="const", bufs=1))
        loads = ctx.enter_context(tc.tile_pool(name="loads", bufs=4))
        acts = ctx.enter_context(tc.tile_pool(name="acts", bufs=4))
        ms = ctx.enter_context(tc.tile_pool(name="ms", bufs=3))
        logms = ctx.enter_context(tc.tile_pool(name="logms", bufs=3))
        outs = ctx.enter_context(tc.tile_pool(name="outs", bufs=1))
        psum = ctx.enter_context(tc.tile_pool(name="psum", bufs=1, space="PSUM"))

        ones_sb = const.tile([P, 1], fp16)
        nc.vector.memset(ones_sb[:], 2.0 ** -16)
        nones_sb = const.tile([P, 1], fp16)
        nc.vector.memset(nones_sb[:], -(2.0 ** -16))
        neg_ln2 = const.tile([P, 1], fp32)
        nc.vector.memset(neg_ln2[:], -LN2)

        acc1 = psum.tile([1, R], fp32, tag="acc1")
        acc2 = psum.tile([1, R], fp32, tag="acc2")

        # software-pipeline queues: Ln lags one super-tile behind Exp
        # (a tile's Ln is emitted right after the NEXT tile's Exp, so a
        # late m never head-of-line blocks the ACT queue)
        ln_q = []  # groups: lists of subs
        prod2_q = []
        nred1 = 0
        nred2 = 0

        def emit_ln(group):
            mbuf = group[0]["m"]
            lnbuf = group[0]["lnm"]
            tot = sum(s["w"] for s in group)
            off = group[0]["moff"]
            nc.scalar.activation(
                out=lnbuf[:, off : off + tot],
                in_=mbuf[:, off : off + tot],
                func=Ln,
            )
            prod2_q.extend(group)

        def emit_prod1(sub, on_pool=False):
            # prod1 = pq * lplq, in-place over lplq (fp16 SBUF -> 2x).
            # No Ln dependency: frees the load buffer one Exp later.
            nonlocal nred1
            w = sub["w"]
            lplq_, pq_ = sub["lplq"], sub["pq"]
            eng = nc.gpsimd if on_pool else nc.vector
            eng.tensor_tensor(
                out=lplq_[:, 0 : 2 * w],
                in0=lplq_[:, 0 : 2 * w],
                in1=pq_[:, 0 : 2 * w],
                op=mult,
            )
            for j in range(0, 2 * w, R):
                nc.tensor.matmul(
                    out=acc1[:, 0:R],
                    lhsT=ones_sb[:],
                    rhs=lplq_[:, j : j + R],
                    start=(nred1 == 0),
                    stop=(nred1 == NRED - 1),
                )
                nred1 += 1

        def emit_prod2(sub):
            # prod2 = pq * lnm (lnm broadcast over both pair halves via a
            # zero-stride middle dim), in-place over pq.
            nonlocal nred2
            w = sub["w"]
            pq_ = sub["pq"]
            off = sub["moff"]
            g_ = w // R
            pair = pq_[:, 0 : 2 * w].rearrange(
                "p (b a t) -> p b a t", b=g_, a=2
            )
            lnb = (
                sub["lnm"][:, off : off + w]
                .rearrange("p (b t) -> p b t", b=g_)
                .unsqueeze(2)
                .broadcast_to([P, g_, 2, R])
            )
            nc.vector.tensor_tensor(out=pair, in0=pair, in1=lnb, op=mult)
            for j in range(0, 2 * w, R):
                nc.tensor.matmul(
                    out=acc2[:, 0:R],
                    lhsT=nones_sb[:],
                    rhs=pq_[:, j : j + R],
                    start=(nred2 == 0),
                    stop=(nred2 == NRED - 1),
                )
                nred2 += 1

        def emit_dma(lplq, v0, g, w):
            # DRAM [((b a) p) t] -> SBUF [p (b a) t]: one DMA per pair
            # tile; SBUF pair layout is block-interleaved [lp_b|lq_b|...]
            r0 = 2 * v0
            src = lpq[r0 : r0 + 2 * g * P, :].rearrange(
                "(c p) t -> p c t", p=P
            )
            dst = lplq[:, 0 : 2 * w].rearrange("p (c t) -> p c t", c=2 * g)
            nc.sync.dma_start(out=dst, in_=src)

        # the last NTAIL tiles get dedicated (non-rotating) load buffers,
        # DMA'd mid-stream so the drain phase never waits on loads; they
        # also get dedicated pq buffers so their Exps never wait on the
        # PE reduce chain recycling the rotating pq pool
        NTAIL = 4
        tail_tiles = {}
        tail_pqs = {}
        tail_v0 = V - sum(TILES[-NTAIL:]) * P
        PREFETCH_AT = len(TILES) - NTAIL - 6

        # pair consecutive full-size body tiles into Ln groups
        pair_start = {}
        pair_second = set()

        v0 = 0
        cur_group = None
        for si, g in enumerate(TILES):
            w = g * R
            if si == PREFETCH_AT:
                tv = tail_v0
                for ti, tg in enumerate(TILES[-NTAIL:]):
                    tw = tg * R
                    tl = const.tile([P, 2 * tw], fp16, tag=f"tail{ti}",
                                    name=f"tail{ti}")
                    emit_dma(tl, tv, tg, tw)
                    tail_tiles[len(TILES) - NTAIL + ti] = tl
                    tail_pqs[len(TILES) - NTAIL + ti] = const.tile(
                        [P, 2 * tw], fp16, tag=f"tpq{ti}", name=f"tpq{ti}"
                    )
                    tv += tg * P
            if si in tail_tiles:
                lplq = tail_tiles[si]
                pq = tail_pqs[si]
            else:
                lplq = loads.tile([P, 2 * WMAX], fp16, tag="lplq", name="lplq")
                emit_dma(lplq, v0, g, w)
                pq = acts.tile([P, 2 * WMAX], fp16, tag="pq", name="pq")
            m = ms.tile([P, WMAX], fp16, tag="m", name="m")
            lnm = logms.tile([P, WMAX], fp16, tag="lnm", name="lnm")
            moff = 0
            # ACT: Exp(s); Ln(s-1) is emitted right after so the ACT
            # stream is Exp(0) Exp(1) Ln(0) Exp(2) Ln(1) ...
            nc.scalar.activation(
                out=pq[:, 0 : 2 * w],
                in_=lplq[:, 0 : 2 * w],
                func=Exp,
                scale=1.0,
                bias=neg_ln2[:],
            )
            # m = p' + q' split Pool/DVE so both finish inside the next
            # Exp window; near the taper the window shrinks, so cap the
            # Pool share by the next tile's Exp duration (1.98 ns/elem on
            # Pool vs 0.833 ns/col of Exp ahead of the dependent Ln).
            # pq is block-interleaved [p (b a t)]: lp at a=0, lq at a=1
            pq3 = pq[:, 0 : 2 * w].rearrange("p (b a t) -> p b a t", b=g, a=2)
            m3 = m[:, moff : moff + w].rearrange("p (b t) -> p b t", b=g)
            g_next = TILES[si + 1] if si + 1 < len(TILES) else 0
            wp_win = max(0, int(1.667 * g_next * R - 200) * 100 // 198)
            bp = min(min(int(w * POOL_SHARE), wp_win) // R, g)
            if si < 5:
                # during DMA-limited fill, DVE is idle and Pool lag
                # cascades into the first Lns
                bp = 0
            elif si >= NT - 12:
                # during wind-down DVE is draining prod backlog while
                # Pool is idle: put m fully on Pool
                bp = g
            if bp > 0:
                nc.gpsimd.tensor_tensor(
                    out=m3[:, 0:bp],
                    in0=pq3[:, 0:bp, 0],
                    in1=pq3[:, 0:bp, 1],
                    op=add,
                )
            if bp < g:
                nc.vector.tensor_tensor(
                    out=m3[:, bp:g],
                    in0=pq3[:, bp:g, 0],
                    in1=pq3[:, bp:g, 1],
                    op=add,
                )
            sub = {"lplq": lplq, "pq": pq, "m": m, "lnm": lnm, "w": w,
                   "moff": moff}
            emit_prod1(sub)
            while ln_q:
                emit_ln(ln_q.pop(0))
            while prod2_q:
                emit_prod2(prod2_q.pop(0))
            if si in pair_start:
                cur_group = [sub]
            elif si in pair_second:
                cur_group.append(sub)
                ln_q.append(cur_group)
                cur_group = None
            else:
                ln_q.append([sub])
            v0 += g * P
        assert v0 == V
        while ln_q:
            emit_ln(ln_q.pop(0))
        while prod2_q:
            emit_prod2(prod2_q.pop(0))
        assert nred1 == NRED and nred2 == NRED

        # finale: the +/-2^-16 lhsT (exact fp16 subnormal) already applied
        # the full 2^-(B+1) scale and the sign; loss = acc1 + acc2
        loss_sb = outs.tile([1, R], fp32, tag="lb", name="loss_sb")
        nc.vector.tensor_scalar_mul(out=loss_sb[:], in0=acc1[:], scalar1=1.0)
        nc.vector.tensor_tensor(
            out=loss_sb[:], in0=loss_sb[:], in1=acc2[:], op=add
        )
        nc.sync.dma_start(out=out_d.ap(), in_=loss_sb[:])

    nc.compile()
    return nc


def _get_program():
    if "nc" not in _CACHE:
        _CACHE["nc"] = _build_program()
    return _CACHE["nc"]


def kernel(log_q: np.ndarray, log_p: np.ndarray, _trace: bool = False):
    from concourse.bass_utils import run_bass_kernel_spmd

    log_q = np.asarray(log_q, dtype=np.float32)
    log_p = np.asarray(log_p, dtype=np.float32)
    assert log_q.shape == (N, V) and log_p.shape == (N, V)

    lp3 = (log_p + (BIAS + 1.0) * LN2).astype(np.float16).T
    lq3 = (log_q + (BIAS + 1.0) * LN2).astype(np.float16).T
    # block-interleaved rows: (vblock, a, p) -> row (vblock*2 + a)*128 + p
    lpq3 = np.ascontiguousarray(
        np.stack(
            [lp3.reshape(NVB, P, N), lq3.reshape(NVB, P, N)], axis=1
        ).reshape(2 * V, N)
    )

    nc = _get_program()
    in_maps = []
    for c in range(NCORES):
        sl = slice(c * R, (c + 1) * R)
        in_maps.append({"lpq3": np.ascontiguousarray(lpq3[:, sl])})
    res = run_bass_kernel_spmd(
        nc, in_maps, core_ids=list(range(NCORES)), trace=_trace
    )
    _CACHE["last_results"] = res
    outs = [res.results[c]["loss"].reshape(R) for c in range(NCORES)]
    return np.concatenate(outs, axis=0).astype(np.float32)
